# revision 5
# baseline (speedup 1.0000x reference)
"""Trainium2 Bass kernel for nn_ExpandedSiameseMerge.

Problem: N=2048 anchors, D=128 feats, C=64 classes, K=32 rows/class.
Each anchor emits 94 pair rows ([anchor_feat | other_feat], 2D=256 f32):
31 positives (own class block minus self) + 63 negatives (one random row
from each other class, in class order). Output: [192512, 256] f32 plus
the deterministic labels vector [192512] int32.

Strategy (data-parallel over anchors, 8 cores, 256 anchors/core):
  - Host computes the [N, 94] pair-index table from targets/neg_rand
    (cheap numpy; identical formulas to the reference).
  - Each core runs 4 double-buffered chunks. A chunk covers 64 anchors
    = 6016 output rows = 47 rows per SBUF partition (94 = 2*47, so each
    partition holds exactly half of one anchor's rows).
  - Per chunk one SWDGE dma_gather (InstDMAGatherAnt) pulls 12032
    512B-elements from the replicated curr table in HBM straight into
    the interleaved [left|right] layout (gather element i lands at
    partition i%128, slot i//128; the host permutes the index array so
    that each partition's 47 output rows are contiguous).
  - One HWDGE dma_start per chunk writes the finished [128 x 48128B]
    tile back to HBM as one contiguous 48KB descriptor per partition.
"""

import numpy as np

import concourse.bacc as bacc
import concourse.bass as bass
import concourse.mybir as mybir
from concourse._compat import get_trn_type
from concourse.bass_utils import run_bass_kernel_spmd
from concourse.library_config import mlp

N, D, C = 2048, 128, 64
K = N // C                 # 32 rows per class block
PP = (K - 1) + (C - 1)     # 94 pair rows per anchor
M = 8                      # cores
APC = N // M               # 256 anchors per core
RPC = APC * PP             # 24064 output rows per core
NCHUNK = 4                 # chunks per core
Q = PP // 2                # 47 output rows per partition per chunk
CHROWS = 128 * Q           # 6016 output rows per chunk
NIDX = 2 * CHROWS          # 12032 gather elements per chunk (left+right)
IDXCOLS = NIDX // 16       # 752 int16 per idx partition per chunk

_prog_cache = None


def _build_program():
    nc = bacc.Bacc(get_trn_type() or "TRN2")
    curr_d = nc.dram_tensor("curr", [N, D], mybir.dt.float32, kind="ExternalInput")
    gidx_d = nc.dram_tensor(
        "gidx", [128, NCHUNK * IDXCOLS], mybir.dt.int16, kind="ExternalInput"
    )
    out_d = nc.dram_tensor(
        "out", [NCHUNK, 128, Q * 2 * D], mybir.dt.float32, kind="ExternalOutput"
    )
    with (
        nc.Block() as block,
        nc.sbuf_tensor("buf", [128, 2, PP, D], mybir.dt.float32) as buf,
        nc.sbuf_tensor("idxs", [128, NCHUNK * IDXCOLS], mybir.dt.int16) as idxs,
        nc.semaphore("ld") as ld,
        nc.semaphore("gs") as gs,
        nc.semaphore("os") as osem,
    ):

        @block.gpsimd
        def _(g: bass.BassGpSimd):
            g.load_library(mlp)
            g.dma_start(out=idxs[:], in_=gidx_d[:]).then_inc(ld, 16)
            g.wait_ge(ld, 16)
            for k in range(NCHUNK):
                if k >= 2:
                    # buffer k%2 was last read by out-DMA k-2
                    g.wait_ge(osem, 16 * (k - 1))
                g.dma_gather(
                    buf[:, k % 2],
                    curr_d[:],
                    idxs[:, k * IDXCOLS : (k + 1) * IDXCOLS],
                    NIDX,
                    NIDX,
                    D,
                    single_packet=False,
                ).then_inc(gs, 16)

        @block.sync
        def _(s: bass.BassEngine):
            for k in range(NCHUNK):
                s.wait_ge(gs, 16 * (k + 1))
                s.dma_start(
                    out=out_d[k], in_=buf[:, k % 2].rearrange("p g e -> p (g e)")
                ).then_inc(osem, 16)
            s.wait_ge(osem, 16 * NCHUNK)

    nc.compile()
    return nc


def _get_program():
    global _prog_cache
    if _prog_cache is None:
        _prog_cache = _build_program()
    return _prog_cache


def _pair_table(targets, neg_rand):
    """[N, 94] int64: reference's concat([pos_idx, neg_idx], axis=1)."""
    cls = targets.astype(np.int64)
    row = np.arange(N, dtype=np.int64) % K
    base_p = np.arange(K - 1, dtype=np.int64)[None, :]
    pos_j = base_p + (base_p >= row[:, None])
    pos_idx = cls[:, None] * K + pos_j
    base_n = np.arange(C - 1, dtype=np.int64)[None, :]
    neg_cls = base_n + (base_n >= cls[:, None])
    neg_idx = neg_cls * K + neg_rand.astype(np.int64)
    return np.concatenate([pos_idx, neg_idx], axis=1)


def _gidx_for_core(pair, c):
    """[128, NCHUNK*752] int16 wrapped gather indices for core c.

    Gather element i of chunk k lands at partition i%128, slot i//128.
    We map (row j, half h, partition p) -> i = (2j+h)*128 + p so that
    partition p holds output rows k*6016 + p*47 .. +47 contiguously.
    The SWDGE reads idxs wrapped over 16 partitions (idx i at
    [i%16, i//16]), replicated across the 8 Q7 core stripes.
    """
    j = np.arange(Q, dtype=np.int64)
    p = np.arange(128, dtype=np.int64)
    cols = []
    for k in range(NCHUNK):
        rl = k * CHROWS + p[:, None] * Q + j[None, :]   # [128, 47] core-local row
        anchor = c * APC + rl // PP
        right = pair[anchor, rl % PP]
        arr = np.empty((Q, 2, 128), np.int16)           # [j, h, p]
        arr[:, 0, :] = anchor.T
        arr[:, 1, :] = right.T
        lin = arr.reshape(-1)                           # lin[(2j+h)*128+p]
        wrapped = lin.reshape(IDXCOLS, 16).T            # [16, 752]
        cols.append(np.tile(wrapped, (8, 1)))           # [128, 752]
    return np.ascontiguousarray(np.concatenate(cols, axis=1))


def _labels():
    lab = np.concatenate(
        [np.ones(K - 1, np.int32), np.zeros(C - 1, np.int32)]
    )
    return np.tile(lab, N)


def _run(curr, targets, neg_rand, trace=False):
    curr = np.ascontiguousarray(np.asarray(curr, dtype=np.float32))
    targets = np.asarray(targets)
    neg_rand = np.asarray(neg_rand)
    pair = _pair_table(targets, neg_rand)
    in_maps = [{"curr": curr, "gidx": _gidx_for_core(pair, c)} for c in range(M)]
    nc = _get_program()
    res = run_bass_kernel_spmd(nc, in_maps, list(range(M)), trace=trace)
    outs = [np.asarray(r["out"]).reshape(RPC, 2 * D) for r in res.results]
    expanded = np.concatenate(outs, axis=0)
    return (expanded, _labels()), res


def kernel(curr, targets, neg_rand, num_classes):
    assert int(num_classes) == C
    out, _ = _run(curr, targets, neg_rand, trace=False)
    return out


# revision 6
# speedup vs baseline: 2.9269x; 2.9269x over previous
"""Plan E: PE one-hot matmul expansion kernel (no DMA gather).

Per core: 256 anchors in 2 chunks of 128 (anchor = partition).
- Negatives: per class c, psum = oh_c^T @ block_c gives each anchor its
  selected class-c row; two predicated copies place it at compacted slot
  30+c or 31+c (skip own class).
- Positives: per block-row j, psum = ohc^T @ currstr_j gives each anchor
  row j of its own class block; predicated copies compact around the
  self row (slot j or j-1), and the j==self tile provides the anchor's
  own row (the left half source).
- Output: rights tile [128, 94, 128] + broadcast left halves, written
  with strided DMAs into the final [rows, 256] layout.
"""

import numpy as np

import concourse.bacc as bacc
import concourse.bass as bass
import concourse.mybir as mybir
import concourse.tile as tile
from concourse._compat import get_trn_type
from concourse.bass_utils import run_bass_kernel_spmd

F32 = mybir.dt.float32
I8 = mybir.dt.int8

N, D, C = 2048, 128, 64
K = N // C                 # 32
PP = (K - 1) + (C - 1)     # 94
M = 8
APC = N // M               # 256 anchors/core
RPC = APC * PP             # 24064 rows/core
NCH = 2                    # chunks per core
CH = APC // NCH            # 128 anchors per chunk

_prog_cache = None


def _build_program():
    nc = bacc.Bacc(get_trn_type() or "TRN2", target_bir_lowering=False)
    blocks32_d = nc.dram_tensor("blocks32", [K, C * D], F32, kind="ExternalInput")
    currstr_d = nc.dram_tensor("currstr", [C, K * D], F32, kind="ExternalInput")
    oh_d = nc.dram_tensor("oh", [NCH, K, C * D], F32, kind="ExternalInput")
    ohc_d = nc.dram_tensor("ohc", [NCH, C, CH], F32, kind="ExternalInput")
    mgt_d = nc.dram_tensor("mgt", [NCH, CH, C], I8, kind="ExternalInput")
    mlt_d = nc.dram_tensor("mlt", [NCH, CH, C], I8, kind="ExternalInput")
    mplt_d = nc.dram_tensor("mplt", [CH, K], I8, kind="ExternalInput")
    mpgt_d = nc.dram_tensor("mpgt", [CH, K], I8, kind="ExternalInput")
    mself_d = nc.dram_tensor("mself", [CH, K], I8, kind="ExternalInput")
    out_d = nc.dram_tensor("out", [NCH, CH, PP, 2 * D], F32, kind="ExternalOutput")

    with tile.TileContext(nc) as tc:
        with (
            tc.tile_pool(name="const", bufs=1) as cpool,
            tc.tile_pool(name="oh", bufs=1) as ohpool,
            tc.tile_pool(name="msk", bufs=2) as mpool,
            tc.tile_pool(name="R", bufs=2) as rpool,
            tc.tile_pool(name="ps", bufs=8, space="PSUM") as ps,
        ):
            blocks32 = cpool.tile([K, C * D], F32)
            nc.sync.dma_start(blocks32[:], blocks32_d[:])
            currstr = cpool.tile([C, K * D], F32)
            nc.sync.dma_start(currstr[:], currstr_d[:])
            mplt = cpool.tile([CH, K], I8)
            nc.sync.dma_start(mplt[:], mplt_d[:])
            mpgt = cpool.tile([CH, K], I8)
            nc.sync.dma_start(mpgt[:], mpgt_d[:])
            mself = cpool.tile([CH, K], I8)
            nc.sync.dma_start(mself[:], mself_d[:])

            for k in range(NCH):
                oh = ohpool.tile([K, C * D], F32, tag="oh")
                nc.sync.dma_start(oh[:], oh_d[k])
                ohc = ohpool.tile([C, CH], F32, tag="ohc")
                nc.sync.dma_start(ohc[:], ohc_d[k])
                mgt = mpool.tile([CH, C], I8, tag="mgt")
                nc.sync.dma_start(mgt[:], mgt_d[k])
                mlt = mpool.tile([CH, C], I8, tag="mlt")
                nc.sync.dma_start(mlt[:], mlt_d[k])

                R = rpool.tile([CH, PP, D], F32, tag="R")
                L = rpool.tile([CH, D], F32, tag="L")

                # positives + left halves: 32 matmuls against currstr
                for j in range(K):
                    pt = ps.tile([CH, D], F32, space="PSUM", tag="ppos")
                    nc.tensor.matmul(
                        pt[:], ohc[:], currstr[:, j * D : (j + 1) * D],
                        start=True, stop=True,
                    )
                    if j < K - 1:
                        nc.vector.copy_predicated(
                            R[:, j, :], mplt[:, j : j + 1].to_broadcast([CH, D]), pt[:]
                        )
                    if j > 0:
                        nc.vector.copy_predicated(
                            R[:, j - 1, :],
                            mpgt[:, j : j + 1].to_broadcast([CH, D]),
                            pt[:],
                        )
                    nc.vector.copy_predicated(
                        L[:], mself[:, j : j + 1].to_broadcast([CH, D]), pt[:]
                    )

                # negatives: 64 matmuls against blocks32
                for c in range(C):
                    pt = ps.tile([CH, D], F32, space="PSUM", tag="pneg")
                    nc.tensor.matmul(
                        pt[:],
                        oh[:, c * D : (c + 1) * D],
                        blocks32[:, c * D : (c + 1) * D],
                        start=True, stop=True,
                    )
                    if c > 0:
                        nc.vector.copy_predicated(
                            R[:, K - 2 + c, :],
                            mgt[:, c : c + 1].to_broadcast([CH, D]),
                            pt[:],
                        )
                    if c < C - 1:
                        nc.vector.copy_predicated(
                            R[:, K - 1 + c, :],
                            mlt[:, c : c + 1].to_broadcast([CH, D]),
                            pt[:],
                        )

                nc.sync.dma_start(out_d[k, :, :, D : 2 * D], R[:])
                nc.sync.dma_start(
                    out_d[k, :, :, 0:D],
                    L[:].unsqueeze(1).to_broadcast([CH, PP, D]),
                )

    nc.compile()
    return nc


def _get_program():
    global _prog_cache
    if _prog_cache is None:
        _prog_cache = _build_program()
    return _prog_cache


def _host_prep(curr, targets, neg_rand):
    """Returns (shared_inputs, per_core_inputs list)."""
    curr = np.ascontiguousarray(np.asarray(curr, dtype=np.float32))
    cls = np.asarray(targets).astype(np.int64)
    neg_rand = np.asarray(neg_rand).astype(np.int64)

    blocks32 = np.ascontiguousarray(
        curr.reshape(C, K, D).transpose(1, 0, 2).reshape(K, C * D)
    )
    currstr = np.ascontiguousarray(curr.reshape(C, K * D))

    s1 = np.arange(CH, dtype=np.int64) % K
    j = np.arange(K, dtype=np.int64)
    mplt = (j[None, :] < s1[:, None]).astype(np.int8)
    mpgt = (j[None, :] > s1[:, None]).astype(np.int8)
    mself = (j[None, :] == s1[:, None]).astype(np.int8)

    # negative selected row per (anchor, class): neg_rand[a, c - (c > cls)] or 0
    carr = np.arange(C, dtype=np.int64)
    percore = []
    for core in range(M):
        oh = np.zeros((NCH, K, C, CH), np.float32)
        ohc = np.zeros((NCH, C, CH), np.float32)
        mgt = np.zeros((NCH, CH, C), np.int8)
        mlt = np.zeros((NCH, CH, C), np.int8)
        for k in range(NCH):
            a = core * APC + k * CH + np.arange(CH)
            ca = cls[a]                                   # [CH]
            negslot = carr[None, :] - (carr[None, :] > ca[:, None])  # [CH, C]
            sel = neg_rand[a[:, None], negslot]           # [CH, C]
            sel[carr[None, :] == ca[:, None]] = 0         # dummy for own class
            # oh[k, r, c, m] = (sel[m, c] == r)
            oh[k, sel.T, carr[:, None], np.arange(CH)[None, :]] = 1.0
            ohc[k, ca, np.arange(CH)] = 1.0
            mgt[k] = (carr[None, :] > ca[:, None]).astype(np.int8)
            mlt[k] = (carr[None, :] < ca[:, None]).astype(np.int8)
        percore.append(
            {
                "oh": np.ascontiguousarray(oh.reshape(NCH, K, C * D)),
                "ohc": np.ascontiguousarray(ohc),
                "mgt": mgt,
                "mlt": mlt,
            }
        )
    shared = {
        "blocks32": blocks32,
        "currstr": currstr,
        "mplt": mplt,
        "mpgt": mpgt,
        "mself": mself,
    }
    return shared, percore


def _labels():
    lab = np.concatenate([np.ones(K - 1, np.int32), np.zeros(C - 1, np.int32)])
    return np.tile(lab, N)


def _run(curr, targets, neg_rand, trace=False, cores=None):
    shared, percore = _host_prep(curr, targets, neg_rand)
    core_ids = list(range(M)) if cores is None else cores
    in_maps = [{**shared, **percore[c]} for c in core_ids]
    nc = _get_program()
    res = run_bass_kernel_spmd(nc, in_maps, core_ids, trace=trace)
    outs = [np.asarray(r["out"]).reshape(RPC, 2 * D) for r in res.results]
    expanded = np.concatenate(outs, axis=0)
    return (expanded, _labels()), res


def kernel(curr, targets, neg_rand, num_classes):
    assert int(num_classes) == C
    out, _ = _run(curr, targets, neg_rand, trace=False)
    return out


# revision 7
# speedup vs baseline: 3.0574x; 1.0446x over previous
"""Plan F: bf16 3-plane one-hot matmul expansion kernel.

curr rows are split on the host into three non-overlapping bf16 planes
(truncated mantissa ranges, x = h1+h2+h3 exactly; the PE's in-order
systolic accumulation reconstructs fp32 bit-exactly).

Per core: 256 anchors in 2 chunks of 128 (anchor = psum partition).
- Negatives per output slot s (0..62): the slot draws from class s or
  s+1 only, so one K=128 matmul (2 planes x 64 rows of the class pair)
  + one K=64 accumulate (3rd plane) yields the slot row for all 128
  anchors, with the own-class skip folded into the one-hot. Results
  land in psum groups of 4 slots and are plain-copied into R.
- Positives per block row j (0..31): K=128 (2 planes x 64 classes) +
  K=64 accumulate gives row j of each anchor's own block; predicated
  copies compact around the self row.
- Left halves arrive via a host-sliced DMA and are broadcast by a
  zero-stride output DMA.
"""

import numpy as np
import ml_dtypes

import concourse.bacc as bacc
import concourse.bass as bass
import concourse.mybir as mybir
import concourse.tile as tile
from concourse._compat import get_trn_type
from concourse.bass_utils import run_bass_kernel_spmd

F32 = mybir.dt.float32
BF16 = mybir.dt.bfloat16
I8 = mybir.dt.int8

N, D, C = 2048, 128, 64
K = N // C                 # 32
PP = (K - 1) + (C - 1)     # 94
NSLOT = C - 1              # 63 negative slots
M = 8
APC = N // M               # 256 anchors/core
RPC = APC * PP             # 24064 rows/core
NCH = 2
CH = APC // NCH            # 128 anchors per chunk
NEG_EVEN = 32              # even slots 0,2,..,62
NEG_ODD = 31               # odd slots 1,3,..,61

_prog_cache = None


def _trunc_bf16(x):
    m = x.view(np.uint32) & np.uint32(0xFFFF0000)
    return m.astype(np.uint32).view(np.float32)


def _split3(x):
    h1 = _trunc_bf16(x)
    r = x - h1
    h2 = _trunc_bf16(r)
    h3 = r - h2
    assert np.array_equal(h3, _trunc_bf16(h3))
    assert np.array_equal(h1 + h2 + h3, x)
    bf = ml_dtypes.bfloat16
    return h1.astype(bf), h2.astype(bf), h3.astype(bf)


def _build_program():
    nc = bacc.Bacc(get_trn_type() or "TRN2", target_bir_lowering=False)
    # shared (curr-derived) sources
    bnegA_d = nc.dram_tensor("bnegA", [128, NEG_EVEN * D], BF16, kind="ExternalInput")
    bnegB_d = nc.dram_tensor("bnegB", [128, NEG_ODD * D], BF16, kind="ExternalInput")
    bnegA3_d = nc.dram_tensor("bnegA3", [64, NEG_EVEN * D], BF16, kind="ExternalInput")
    bnegB3_d = nc.dram_tensor("bnegB3", [64, NEG_ODD * D], BF16, kind="ExternalInput")
    bposA_d = nc.dram_tensor("bposA", [128, K * D], BF16, kind="ExternalInput")
    bposA3_d = nc.dram_tensor("bposA3", [64, K * D], BF16, kind="ExternalInput")
    mplt_d = nc.dram_tensor("mplt", [CH, K], I8, kind="ExternalInput")
    mpgt_d = nc.dram_tensor("mpgt", [CH, K], I8, kind="ExternalInput")
    # per-core
    lefts_d = nc.dram_tensor("lefts", [NCH, CH, D], F32, kind="ExternalInput")
    ohn_d = nc.dram_tensor("ohn", [NCH, 128, NSLOT * CH], BF16, kind="ExternalInput")
    ohp_d = nc.dram_tensor("ohp", [NCH, 128, CH], BF16, kind="ExternalInput")
    out_d = nc.dram_tensor("out", [NCH, CH, PP, 2 * D], F32, kind="ExternalOutput")

    with tile.TileContext(nc) as tc:
        with (
            tc.tile_pool(name="const", bufs=1) as cpool,
            tc.tile_pool(name="ohs", bufs=2) as ohpool,
            tc.tile_pool(name="R", bufs=2) as rpool,
            tc.tile_pool(name="ps", bufs=8, space="PSUM") as ps,
        ):
            bnegA = cpool.tile([128, NEG_EVEN * D], BF16)
            nc.sync.dma_start(bnegA[:], bnegA_d[:])
            bnegB = cpool.tile([128, NEG_ODD * D], BF16)
            nc.sync.dma_start(bnegB[:], bnegB_d[:])
            bnegA3 = cpool.tile([64, NEG_EVEN * D], BF16)
            nc.sync.dma_start(bnegA3[:], bnegA3_d[:])
            bnegB3 = cpool.tile([64, NEG_ODD * D], BF16)
            nc.sync.dma_start(bnegB3[:], bnegB3_d[:])
            bposA = cpool.tile([128, K * D], BF16)
            nc.sync.dma_start(bposA[:], bposA_d[:])
            bposA3 = cpool.tile([64, K * D], BF16)
            nc.sync.dma_start(bposA3[:], bposA3_d[:])
            mplt = cpool.tile([CH, K], I8)
            nc.sync.dma_start(mplt[:], mplt_d[:])
            mpgt = cpool.tile([CH, K], I8)
            nc.sync.dma_start(mpgt[:], mpgt_d[:])

            for k in range(NCH):
                ohn = ohpool.tile([128, NSLOT * CH], BF16, tag="ohn")
                nc.sync.dma_start(ohn[:], ohn_d[k])
                ohp = ohpool.tile([128, CH], BF16, tag="ohp")
                nc.sync.dma_start(ohp[:], ohp_d[k])

                R = rpool.tile([CH, PP, D], F32, tag="R")
                L = rpool.tile([CH, D], F32, tag="L")
                nc.sync.dma_start(L[:], lefts_d[k])

                # --- positives: 8 psum groups of 4 block-rows ---
                for j0 in range(0, K, 4):
                    pt = ps.tile([CH, 4 * D], F32, space="PSUM", tag="pt")
                    for u in range(4):
                        j = j0 + u
                        sl = pt[:, u * D : (u + 1) * D]
                        nc.tensor.matmul(
                            sl, ohp[:], bposA[:, j * D : (j + 1) * D],
                            start=True, stop=False,
                        )
                        nc.tensor.matmul(
                            sl, ohp[0:64], bposA3[:, j * D : (j + 1) * D],
                            start=False, stop=True,
                        )
                    # lt: tile j -> slot j where j < s1   (slots j0..j0+3, j<=30)
                    ulo, uhi = j0, min(j0 + 4, K - 1)
                    if ulo < uhi:
                        nc.vector.copy_predicated(
                            R[:, ulo:uhi, :],
                            mplt[:, ulo:uhi].unsqueeze(2).to_broadcast(
                                [CH, uhi - ulo, D]
                            ),
                            pt[:, (ulo - j0) * D : (uhi - j0) * D].rearrange(
                                "p (g e) -> p g e", e=D
                            ),
                        )
                    # gt: tile j -> slot j-1 where j > s1 (units j>=1)
                    glo = max(j0, 1)
                    ghi = j0 + 4
                    nc.vector.copy_predicated(
                        R[:, glo - 1 : ghi - 1, :],
                        mpgt[:, glo:ghi].unsqueeze(2).to_broadcast(
                            [CH, ghi - glo, D]
                        ),
                        pt[:, (glo - j0) * D : (ghi - j0) * D].rearrange(
                            "p (g e) -> p g e", e=D
                        ),
                    )

                # --- negatives: 16 psum groups of 4 slots ---
                for s0 in range(0, NSLOT, 4):
                    ns = min(4, NSLOT - s0)
                    pt = ps.tile([CH, 4 * D], F32, space="PSUM", tag="pt")
                    for u in range(ns):
                        s = s0 + u
                        sl = pt[:, u * D : (u + 1) * D]
                        lhs = ohn[:, s * CH : (s + 1) * CH]
                        if s % 2 == 0:
                            g = s // 2
                            r12 = bnegA[:, g * D : (g + 1) * D]
                            r3 = bnegA3[:, g * D : (g + 1) * D]
                        else:
                            g = (s - 1) // 2
                            r12 = bnegB[:, g * D : (g + 1) * D]
                            r3 = bnegB3[:, g * D : (g + 1) * D]
                        nc.tensor.matmul(sl, lhs, r12, start=True, stop=False)
                        nc.tensor.matmul(sl, lhs[0:64], r3, start=False, stop=True)
                    dst = R[:, K - 1 + s0 : K - 1 + s0 + ns, :]
                    src = pt[:, : ns * D].rearrange("p (g e) -> p g e", e=D)
                    if (s0 // 4) % 2 == 0:
                        nc.vector.tensor_copy(dst, src)
                    else:
                        nc.scalar.copy(dst, src)

                nc.sync.dma_start(out_d[k, :, :, D : 2 * D], R[:])
                nc.sync.dma_start(
                    out_d[k, :, :, 0:D],
                    L[:].unsqueeze(1).to_broadcast([CH, PP, D]),
                )

    nc.compile()
    return nc


def _get_program():
    global _prog_cache
    if _prog_cache is None:
        _prog_cache = _build_program()
    return _prog_cache


def _host_prep(curr, targets, neg_rand):
    curr = np.ascontiguousarray(np.asarray(curr, dtype=np.float32))
    cls = np.asarray(targets).astype(np.int64)
    neg_rand = np.asarray(neg_rand).astype(np.int64)
    h1, h2, h3 = _split3(curr)          # [N, D] bf16 each

    # negatives sources: class pairs (s, s+1)
    # bnegA[p*64 + r, g*D + f] = plane_p(curr[g*64 + r, f]); g = even pair
    cur12 = np.stack([h1, h2], axis=0)                  # [2, N, D]
    bnegA = cur12.reshape(2, NEG_EVEN, 64, D).transpose(0, 2, 1, 3).reshape(
        128, NEG_EVEN * D
    )
    # odd pairs start at row 32
    oddv = cur12[:, 32 : 32 + NEG_ODD * 64, :]
    bnegB = oddv.reshape(2, NEG_ODD, 64, D).transpose(0, 2, 1, 3).reshape(
        128, NEG_ODD * D
    )
    bnegA3 = h3.reshape(NEG_EVEN, 64, D).transpose(1, 0, 2).reshape(64, NEG_EVEN * D)
    bnegB3 = (
        h3[32 : 32 + NEG_ODD * 64]
        .reshape(NEG_ODD, 64, D)
        .transpose(1, 0, 2)
        .reshape(64, NEG_ODD * D)
    )
    # positives sources: bposA[p*64 + cc, j*D + f] = plane_p(curr[cc*32 + j, f])
    bposA = (
        cur12.reshape(2, C, K, D).transpose(0, 1, 2, 3)  # [2, C, K, D]
        .transpose(0, 1, 2, 3)
    )
    bposA = cur12.reshape(2, C, K, D).transpose(0, 1, 2, 3)
    bposA = np.ascontiguousarray(
        cur12.reshape(2, C, K, D).transpose(0, 1, 2, 3)
    )  # [2, C, K, D]
    bposA = bposA.transpose(0, 1, 2, 3).reshape(2 * C, K, D)  # [(p,cc), K, D]
    bposA = np.ascontiguousarray(bposA.reshape(128, K * D))
    bposA3 = np.ascontiguousarray(h3.reshape(C, K, D).reshape(64, K * D))

    s1 = np.arange(CH, dtype=np.int64) % K
    jj = np.arange(K, dtype=np.int64)
    mplt = (jj[None, :] < s1[:, None]).astype(np.int8)
    mpgt = (jj[None, :] > s1[:, None]).astype(np.int8)

    bf = ml_dtypes.bfloat16
    shared = {
        "bnegA": np.ascontiguousarray(bnegA.astype(bf)),
        "bnegB": np.ascontiguousarray(bnegB.astype(bf)),
        "bnegA3": np.ascontiguousarray(bnegA3.astype(bf)),
        "bnegB3": np.ascontiguousarray(bnegB3.astype(bf)),
        "bposA": bposA.astype(bf),
        "bposA3": bposA3.astype(bf),
        "mplt": mplt,
        "mpgt": mpgt,
    }

    sarr = np.arange(NSLOT, dtype=np.int64)
    marr = np.arange(CH, dtype=np.int64)
    percore = []
    for core in range(M):
        ohn = np.zeros((NCH, NSLOT, 128, CH), np.float32)
        ohp = np.zeros((NCH, 128, CH), np.float32)
        for k in range(NCH):
            a = core * APC + k * CH + marr
            ca = cls[a]                                    # [CH]
            # slot s: one-hot row = (s >= cls)*32 + neg_rand[a, s], dup at +64
            up = (sarr[:, None] >= ca[None, :]).astype(np.int64) * K  # [NSLOT, CH]
            pos = up + neg_rand[a][:, :NSLOT].T            # [NSLOT, CH]
            ohn[k, sarr[:, None], pos, marr[None, :]] = 1.0
            ohn[k, sarr[:, None], pos + 64, marr[None, :]] = 1.0
            # positives: one-hot of class, dup planes
            ohp[k, ca, marr] = 1.0
            ohp[k, ca + 64, marr] = 1.0
        lefts = curr[core * APC : (core + 1) * APC].reshape(NCH, CH, D)
        percore.append(
            {
                "lefts": np.ascontiguousarray(lefts),
                "ohn": np.ascontiguousarray(
                    ohn.transpose(0, 2, 1, 3).reshape(NCH, 128, NSLOT * CH)
                ).astype(bf),
                "ohp": np.ascontiguousarray(ohp).astype(bf),
            }
        )
    return shared, percore


def _labels():
    lab = np.concatenate([np.ones(K - 1, np.int32), np.zeros(C - 1, np.int32)])
    return np.tile(lab, N)


def _run(curr, targets, neg_rand, trace=False, cores=None):
    shared, percore = _host_prep(curr, targets, neg_rand)
    core_ids = list(range(M)) if cores is None else cores
    in_maps = [{**shared, **percore[c]} for c in core_ids]
    nc = _get_program()
    res = run_bass_kernel_spmd(nc, in_maps, core_ids, trace=trace)
    outs = [np.asarray(r["out"]).reshape(RPC, 2 * D) for r in res.results]
    expanded = np.concatenate(outs, axis=0)
    return (expanded, _labels()), res


def kernel(curr, targets, neg_rand, num_classes):
    assert int(num_classes) == C
    out, _ = _run(curr, targets, neg_rand, trace=False)
    return out
